# revision 4
# baseline (speedup 1.0000x reference)
import math
import sys

import numpy as np

sys.path.insert(0, "/opt/trn_rl_repo")

from contextlib import ExitStack

import concourse.bass as bass  # noqa: F401
import concourse.tile as tile
from concourse import bacc, mybir
from concourse.bass_utils import run_bass_kernel_spmd
from concourse.masks import make_identity, make_upper_triangular

B, H, S, D = 2, 16, 2048, 128
N_CORES = 8
HPC = (B * H) // N_CORES  # heads per core = 4
NQ = S // 128  # 16 q/k tiles of 128
SCALE = 1.0 / math.sqrt(float(D))
TANH_SCALE = 50.0
F32 = mybir.dt.float32


def _build_nc():
    nc = bacc.Bacc(
        "TRN2", target_bir_lowering=False, debug=False, num_devices=N_CORES
    )
    q_d = nc.dram_tensor("q", (HPC, S, D), F32, kind="ExternalInput")
    k_d = nc.dram_tensor("k", (HPC, D, S), F32, kind="ExternalInput")
    v_d = nc.dram_tensor("v", (HPC, S, D), F32, kind="ExternalInput")
    o_d = nc.dram_tensor("o", (HPC, S, D), F32, kind="ExternalOutput")

    with tile.TileContext(nc) as tc, ExitStack() as ctx:
        singles = ctx.enter_context(tc.tile_pool(name="singles", bufs=1))
        heads = ctx.enter_context(tc.tile_pool(name="heads", bufs=2))
        sb = ctx.enter_context(tc.tile_pool(name="sb", bufs=4))
        outp = ctx.enter_context(tc.tile_pool(name="outp", bufs=4))
        ps_s = ctx.enter_context(tc.tile_pool(name="ps_s", bufs=3, space="PSUM"))
        ps_o = ctx.enter_context(tc.tile_pool(name="ps_o", bufs=2, space="PSUM"))
        ps_t = ctx.enter_context(tc.tile_pool(name="ps_t", bufs=2, space="PSUM"))

        ident = singles.tile([128, 128], F32)
        make_identity(nc, ident)
        # umask[x, y] = 1.0 where x <= y else 0.0 ; in s_T[k, sq] layout the
        # causal-valid region is k <= sq.
        umask = singles.tile([128, 128], F32)
        make_upper_triangular(nc, umask, val=1.0, diag=True)

        for h in range(HPC):
            # K head: [D, S] contiguous in DRAM, lands directly as matmul lhsT.
            k_sb = heads.tile([128, S], F32, tag="k")
            nc.default_dma_engine.dma_start(out=k_sb, in_=k_d[h, :, :])

            # V head as NQ blocks of [128, D+1]; col D is 1.0 so PV matmul also
            # accumulates the softmax denominator.
            v_sb = heads.tile([128, NQ, D + 1], F32, tag="v")
            nc.vector.memset(v_sb, 1.0)
            for j in range(NQ):
                nc.default_dma_engine.dma_start(
                    out=v_sb[:, j, :D], in_=v_d[h, j * 128 : (j + 1) * 128, :]
                )

            # Q head transposed to [D, S] via PE transposes.
            qT = heads.tile([128, S], F32, tag="qT")
            for i in range(NQ):
                q_in = sb.tile([128, 128], F32, tag="qin")
                nc.default_dma_engine.dma_start(
                    out=q_in, in_=q_d[h, i * 128 : (i + 1) * 128, :]
                )
                q_ps = ps_t.tile([128, 128], F32, tag="qps")
                nc.tensor.transpose(q_ps, q_in, ident)
                nc.vector.tensor_copy(qT[:, i * 128 : (i + 1) * 128], q_ps)

            for i in range(NQ):
                acc = ps_o.tile([128, D + 1], F32, tag="acc")
                for j in range(i + 1):
                    s_t = ps_s.tile([128, 128], F32, tag="st")
                    nc.tensor.matmul(
                        s_t,
                        k_sb[:, j * 128 : (j + 1) * 128],
                        qT[:, i * 128 : (i + 1) * 128],
                        start=True,
                        stop=True,
                    )
                    t_t = sb.tile([128, 128], F32, tag="tt")
                    nc.scalar.activation(
                        t_t, s_t, mybir.ActivationFunctionType.Tanh,
                        scale=SCALE / TANH_SCALE,
                    )
                    p_t = sb.tile([128, 128], F32, tag="pt")
                    nc.scalar.activation(
                        p_t, t_t, mybir.ActivationFunctionType.Exp, scale=TANH_SCALE
                    )
                    if j == i:
                        nc.vector.tensor_mul(p_t, p_t, umask)
                    nc.tensor.matmul(
                        acc, p_t, v_sb[:, j, :], start=(j == 0), stop=(j == i)
                    )
                rec = outp.tile([128, 1], F32, tag="rec")
                nc.vector.reciprocal(rec, acc[:, D : D + 1])
                o_t = outp.tile([128, D], F32, tag="ot")
                nc.scalar.activation(
                    o_t, acc[:, :D], mybir.ActivationFunctionType.Copy, scale=rec
                )
                nc.default_dma_engine.dma_start(
                    out=o_d[h, i * 128 : (i + 1) * 128, :], in_=o_t
                )
    nc.compile()
    return nc


_NC_CACHE = None


def kernel(q: np.ndarray, k: np.ndarray, v: np.ndarray) -> np.ndarray:
    global _NC_CACHE
    if _NC_CACHE is None:
        _NC_CACHE = _build_nc()
    nc = _NC_CACHE

    qf = np.ascontiguousarray(q.reshape(B * H, S, D).astype(np.float32))
    kf = np.ascontiguousarray(k.reshape(B * H, D, S).astype(np.float32))
    vf = np.ascontiguousarray(v.reshape(B * H, S, D).astype(np.float32))

    in_maps = []
    for c in range(N_CORES):
        sl = slice(c * HPC, (c + 1) * HPC)
        in_maps.append({"q": qf[sl], "k": kf[sl], "v": vf[sl]})

    res = run_bass_kernel_spmd(nc, in_maps, core_ids=list(range(N_CORES)))
    out = np.empty((B * H, S, D), dtype=np.float32)
    for c in range(N_CORES):
        out[c * HPC : (c + 1) * HPC] = np.asarray(res.results[c]["o"]).reshape(
            HPC, S, D
        )
    return out.reshape(B, H, S, D)
